# revision 1
# baseline (speedup 1.0000x reference)
"""ARD RBF kernel matrix on 8 TRN2 NeuronCores.

out[n, m] = exp(log_outputscale) * exp(-0.5 * sum_d ((x[n,d] - y[m,d]) / l_d)^2)
with l = exp(log_lengthscale).

Per core (rows of x sharded 8-ways):
    invl[d]   = exp(-log_lengthscale[d])
    xs = x * invl, ys = y * invl        (rounded to fp32r, 11-bit mantissa)
    c[n, m]   = sum_d xs[n,d] * ys[m,d]            } one K=66 fp32r matmul:
    y2[m]     = -0.5 * sum_d ys[m,d]^2  (hi+lo rows)} lhsT=[xs; 1; 1]
    x2[n]     = -0.5 * sum_d xs[n,d]^2 + log_os     -> exact f32 ACT bias
    out[n, m] = Exp(c + y2 + x2)                    -> single ScalarE pass

fp32r streams at ~1 cycle/row (vs 4 for fp32) with 11-bit mantissa; the
y2 row (magnitude ~32) is stored as hi + residual-lo fp32r rows so its
rounding error stays ~1e-6. x2/log_os ride the activation bias in full
fp32. Measured HW rel err ~2e-4 (from the 11-bit rounding of xs/ys).

Inputs are staged host-side in transposed layout ([D, points]) so the
contraction dim lands on SBUF partitions with no on-device transposes.

Schedule shape (engines are in-order FIFOs, so program order matters):
all y input DMAs issue first on the SP ring (nothing queues behind output
DMAs); the FIRST column half's y prep is emitted upfront (fast stream
head), the SECOND half's is woven into the middle of the first half's
sweep — its squares/copies run on DVE (idle during main) and its y2
matmuls slot into the PE queue early, so the output-DMA stream never
gaps at the half boundary. Output DMAs are 2 MiB on the SP ring only
(the issuing sequencer babysits each transfer; the ACT ring stays free
for the exp epilogue); the very first tile ships as two 1 MiB DMAs so
the stream starts as soon as the first two y chunks are prepped. Output
tiles are triple-buffered so ACT produces one tile ahead of the ring.
Mid-sweep y2-prep matmuls borrow row 0 of main-pool PSUM tiles (the
main pool owns all 8 banks). Cost-model makespan ~123us vs ~100us
per-core HBM floor; HW-validated rel err ~2e-4.
"""

import numpy as np

import concourse.bass as bass
import concourse.mybir as mybir
import concourse.tile as tile
from concourse import bacc
from concourse.bass_utils import run_bass_kernel_spmd

N_CORES = 8
N, M, D = 8192, 8192, 64
NSH = N // N_CORES  # 1024 x-rows per core

F32 = mybir.dt.float32
F32R = mybir.dt.float32r
AF = mybir.ActivationFunctionType


def build_nc(nsh=NSH, m=M, d=D, use_f32r=True, n_chunk=2048, prep_chunk=1024):
    """Per-core Bass graph. SPMD: same graph on all 8 cores."""
    nc = bacc.Bacc("TRN2", target_bir_lowering=False)

    xT = nc.dram_tensor("xT", [d, nsh], F32, kind="ExternalInput")
    yT = nc.dram_tensor("yT", [d, m], F32, kind="ExternalInput")
    lls = nc.dram_tensor("log_lengthscale", [d], F32, kind="ExternalInput")
    los = nc.dram_tensor("log_outputscale", [1], F32, kind="ExternalInput")
    out = nc.dram_tensor("out", [nsh, m], F32, kind="ExternalOutput")

    n_tiles = nsh // 128          # x tiles (output partition dim)
    mm_n = 512                    # moving free dim per matmul (one PSUM bank)
    n_sub = n_chunk // mm_n       # matmuls per ACT chunk
    naux = 2                      # y2 hi + lo rows
    half = m // 2                 # column half per outer iteration
    hc = half // n_chunk

    # fp32r: fp32 bits with the low 12 mantissa bits zeroed; streams at
    # ~1 cycle/row (vs 4 for fp32). Writers into matmul operands must
    # declare float32r output so HW rounds on write (BIR verifier rule).
    def mmi(ap):  # matmul input view
        return ap.bitcast(F32R) if use_f32r else ap

    def mmo(ap):  # rounded-writer output view
        return ap.bitcast(F32R) if use_f32r else ap

    with tile.TileContext(nc) as tc:
        with (
            tc.tile_pool(name="const", bufs=1) as cpool,
            tc.tile_pool(name="big", bufs=1) as bpool,
            tc.tile_pool(name="outp", bufs=3) as opool,
        ):
            # ---- hyperparameters ----
            lls_sb = cpool.tile([d, 1], F32, tag="lls")
            nc.sync.dma_start(out=lls_sb[:, :], in_=lls[:].rearrange("(d o) -> d o", o=1))
            los_sb = cpool.tile([1, 1], F32, tag="los")
            nc.sync.dma_start(out=los_sb[:, :], in_=los[:].rearrange("(a o) -> a o", o=1))

            invl = cpool.tile([d, 1], F32, tag="invl")
            nc.scalar.activation(invl[:, :], lls_sb[:, :], AF.Exp, scale=-1.0)
            # -0.5 weight vector for the square-reduce matmuls; consumed by
            # f32r matmuls so it needs an f32r-writing producer (copy).
            neghalf_f = cpool.tile([d, 1], F32, tag="neghalf_f")
            nc.vector.memset(neghalf_f[:, :], -0.5)
            neghalf = cpool.tile([d, 1], F32, tag="neghalf")
            nc.vector.tensor_copy(mmo(neghalf[:, :]), neghalf_f[:, :])
            ones11 = cpool.tile([1, 1], F32, tag="ones11")
            nc.vector.memset(ones11[:, :], 1.0)

            # ---- x side: x_aug = [xs; 1; 1] ----
            # Raw DMA lands in a separate tile: every writer of y_aug/x_aug
            # must carry float32r output dtype (BIR fp32r-producer rule).
            x_raw = bpool.tile([d, nsh], F32, tag="x_raw")
            nc.sync.dma_start(out=x_raw[:, :], in_=xT[:, :])
            x_aug = bpool.tile([d + naux, nsh], F32, tag="x_aug")
            nc.vector.tensor_scalar_mul(mmo(x_aug[0:d, :]), x_raw[:, :], invl[:, :])
            xsq = bpool.tile([d, nsh], F32, tag="xsq")
            nc.vector.tensor_mul(mmo(xsq[:, :]), x_aug[0:d, :], x_aug[0:d, :])
            # memset can't encode an f32r output dtype; copy from an f32 tile
            ones_rows = cpool.tile([naux, nsh], F32, tag="ones_rows")
            nc.vector.memset(ones_rows[:, :], 1.0)
            nc.vector.tensor_copy(mmo(x_aug[d : d + naux, :]), ones_rows[:, :])

            y_aug = bpool.tile([d + naux, m], F32, tag="y_aug")
            y2l_tmp = cpool.tile([1, m], F32, tag="y2l")
            x2row = cpool.tile([1, nsh], F32, tag="x2row")
            bias_sb = cpool.tile([128, n_tiles], F32, tag="bias")

            # x2 bias row + all y prep; own PSUM pool, closed before the
            # main pool claims all 8 banks. All y input DMAs issue first on
            # the SP ring so nothing queues behind output DMAs later.
            nchunks = m // prep_chunk
            with (
                tc.tile_pool(name="yraw_sb", bufs=nchunks) as yrp,
                tc.tile_pool(name="prep_sb", bufs=2) as psb,
            ):
              with tc.tile_pool(name="prep_psum", bufs=2, space="PSUM") as pp:
                y_raws = []
                for jc in range(0, m, prep_chunk):
                    y_raw = yrp.tile([d, prep_chunk], F32, tag="y_raw")
                    nc.sync.dma_start(out=y_raw[:, :], in_=yT[:, jc : jc + prep_chunk])
                    y_raws.append(y_raw)

                for j0 in range(0, nsh, mm_n):
                    w = min(mm_n, nsh - j0)
                    ps = pp.tile([1, mm_n], F32, tag="x2ps")
                    nc.tensor.matmul(
                        ps[:, :w], mmi(neghalf[:, :]), mmi(xsq[:, j0 : j0 + w]),
                        start=True, stop=True,
                    )
                    nc.scalar.activation(
                        x2row[:, j0 : j0 + w], ps[:, :w], AF.Identity,
                        bias=los_sb[:, :],
                    )
                # transpose x2row chunks to per-partition bias cols [128, n_tiles]
                for i in range(n_tiles):
                    ps = pp.tile([128, 1], F32, tag="biasps")
                    nc.tensor.matmul(
                        ps[:, :], x2row[:, i * 128 : (i + 1) * 128], ones11[:, :],
                        start=True, stop=True,
                    )
                    nc.vector.tensor_copy(bias_sb[:, i : i + 1], ps[:, :])

                # ---- y prep for the FIRST column half, upfront: fast
                # head start for the output-DMA stream. Square on ACT (from
                # raw, fused invl scale), y2-hi copy + y2-lo residual on DVE.
                # DVE writes must start at partition {0,32,64,96}: row d+1
                # (partition 65) goes via a partition-0 tmp + DMA on the
                # scalar ring. ----
                for jc in range(0, half, prep_chunk):
                    slc = slice(jc, jc + prep_chunk)
                    y_raw = y_raws[jc // prep_chunk]
                    nc.gpsimd.tensor_scalar_mul(
                        mmo(y_aug[0:d, slc]), y_raw[:, :], invl[:, :]
                    )
                    ysq = psb.tile([d, prep_chunk], F32, tag="ysq")
                    nc.scalar.activation(
                        mmo(ysq[:, :]), y_raw[:, :], AF.Square, scale=invl[:, :]
                    )
                    for j0 in range(0, prep_chunk, mm_n):
                        sl = slice(jc + j0, jc + j0 + mm_n)
                        ps = pp.tile([1, mm_n], F32, tag="y2ps")
                        nc.tensor.matmul(
                            ps[:, :], mmi(neghalf[:, :]), mmi(ysq[:, j0 : j0 + mm_n]),
                            start=True, stop=True,
                        )
                        nc.vector.tensor_copy(mmo(y_aug[d : d + 1, sl]), ps[:, :])
                        if use_f32r:
                            nc.vector.tensor_sub(
                                mmo(y2l_tmp[:, sl]), ps[:, :], y_aug[d : d + 1, sl],
                            )
                        else:
                            nc.vector.memset(y2l_tmp[:, sl], 0.0)
                    nc.scalar.dma_start(
                        out=mmo(y_aug[d + 1 : d + 2, slc]),
                        in_=mmo(y2l_tmp[:, slc]),
                    )

              # ---- main sweep. The SECOND half's y prep is emitted in the
              # middle of the first half's sweep: its y2 matmuls slot into the
              # PE FIFO long before the second half needs them (no mid-stream
              # DMA gap), and its squares run on DVE, which idles during the
              # main sweep, so the ACT exp stream is untouched. ----
              insert_i = max(0, n_tiles - nchunks // 2)
              with tc.tile_pool(name="main_psum", bufs=2, space="PSUM") as mp:
                  for h in range(2):
                      for i in range(n_tiles):
                          if h == 0 and i == insert_i:
                              for jc in range(half, m, prep_chunk):
                                  slc = slice(jc, jc + prep_chunk)
                                  y_raw = y_raws[jc // prep_chunk]
                                  nc.gpsimd.tensor_scalar_mul(
                                      mmo(y_aug[0:d, slc]), y_raw[:, :], invl[:, :]
                                  )
                                  ysq = psb.tile([d, prep_chunk], F32, tag="ysq")
                                  nc.vector.tensor_mul(
                                      mmo(ysq[:, :]), y_aug[0:d, slc], y_aug[0:d, slc]
                                  )
                                  psy = mp.tile([128, n_chunk], F32, tag="mm")
                                  for j0 in range(0, prep_chunk, mm_n):
                                      sl = slice(jc + j0, jc + j0 + mm_n)
                                      pslice = psy[0:1, j0 : j0 + mm_n]
                                      nc.tensor.matmul(
                                          pslice, mmi(neghalf[:, :]),
                                          mmi(ysq[:, j0 : j0 + mm_n]),
                                          start=True, stop=True,
                                      )
                                      nc.vector.tensor_copy(
                                          mmo(y_aug[d : d + 1, sl]), pslice
                                      )
                                      if use_f32r:
                                          nc.vector.tensor_sub(
                                              mmo(y2l_tmp[:, sl]), pslice,
                                              y_aug[d : d + 1, sl],
                                          )
                                      else:
                                          nc.vector.memset(y2l_tmp[:, sl], 0.0)
                                  nc.scalar.dma_start(
                                      out=mmo(y_aug[d + 1 : d + 2, slc]),
                                      in_=mmo(y2l_tmp[:, slc]),
                                  )
                          ot = opool.tile([128, half], F32, tag="ot")
                          for j2 in range(hc):
                              ps = mp.tile([128, n_chunk], F32, tag="mm")
                              for jj in range(n_sub):
                                  col = h * half + j2 * n_chunk + jj * mm_n
                                  nc.tensor.matmul(
                                      ps[:, jj * mm_n : (jj + 1) * mm_n],
                                      mmi(x_aug[:, i * 128 : (i + 1) * 128]),
                                      mmi(y_aug[:, col : col + mm_n]),
                                      start=True, stop=True,
                                  )
                              nc.scalar.activation(
                                  ot[:, j2 * n_chunk : (j2 + 1) * n_chunk],
                                  ps[:, :], AF.Exp, bias=bias_sb[:, i : i + 1],
                              )
                              if h == 0 and i == 0:
                                  # very first tile: ship each exp chunk as
                                  # its own 1 MiB DMA — the first needs only
                                  # the first two y chunks, starting the
                                  # output stream ~5us earlier for one extra
                                  # DMA's fixed cost.
                                  nc.sync.dma_start(
                                      out=out[0:128, j2 * n_chunk : (j2 + 1) * n_chunk],
                                      in_=ot[:, j2 * n_chunk : (j2 + 1) * n_chunk],
                                  )
                          if not (h == 0 and i == 0):
                              nc.sync.dma_start(
                                  out=out[i * 128 : (i + 1) * 128, h * half : (h + 1) * half],
                                  in_=ot[:, :],
                              )
    nc.finalize()
    return nc


_NC_CACHE = {}


def _get_nc():
    if "nc" not in _NC_CACHE:
        _NC_CACHE["nc"] = build_nc()
    return _NC_CACHE["nc"]


def stage_inputs(x, y, log_lengthscale, log_outputscale):
    x = np.ascontiguousarray(np.asarray(x, dtype=np.float32))
    y = np.ascontiguousarray(np.asarray(y, dtype=np.float32))
    lls = np.ascontiguousarray(np.asarray(log_lengthscale, dtype=np.float32))
    los = np.ascontiguousarray(np.asarray(log_outputscale, dtype=np.float32))

    yT = np.ascontiguousarray(y.T)  # [D, M]
    in_maps = []
    for c in range(N_CORES):
        xT_c = np.ascontiguousarray(x[c * NSH : (c + 1) * NSH].T)  # [D, NSH]
        in_maps.append(
            {"xT": xT_c, "yT": yT, "log_lengthscale": lls, "log_outputscale": los}
        )
    return in_maps


def kernel(x, y, log_lengthscale, log_outputscale):
    in_maps = stage_inputs(x, y, log_lengthscale, log_outputscale)
    res = run_bass_kernel_spmd(_get_nc(), in_maps, core_ids=list(range(N_CORES)))
    return np.concatenate([r["out"] for r in res.results], axis=0)



# revision 3
# speedup vs baseline: 7.8696x; 7.8696x over previous
"""ARD RBF kernel matrix on 8 TRN2 NeuronCores.

out[n, m] = exp(log_outputscale) * exp(-0.5 * sum_d ((x[n,d] - y[m,d]) / l_d)^2)
with l = exp(log_lengthscale).

Per core (rows of x sharded 8-ways):
    invl[d]   = exp(-log_lengthscale[d])
    xs = x * invl, ys = y * invl        (rounded to fp32r, 11-bit mantissa)
    c[n, m]   = sum_d xs[n,d] * ys[m,d]            } one K=66 fp32r matmul:
    y2[m]     = -0.5 * sum_d ys[m,d]^2  (hi+lo rows)} lhsT=[xs; 1; 1]
    x2[n]     = -0.5 * sum_d xs[n,d]^2 + log_os     -> exact f32 ACT bias
    out[n, m] = Exp(c + y2 + x2)                    -> single ScalarE pass

fp32r streams at ~1 cycle/row (vs 4 for fp32) with 11-bit mantissa; the
y2 row (magnitude ~32) is stored as hi + residual-lo fp32r rows so its
rounding error stays ~1e-6. x2/log_os ride the activation bias in full
fp32. The exp output is written as bf16 (8-bit exponent covers the
e^-60-scale tail values; ~0.4% elementwise rounding) and upcast to f32
on the host — this halves the output HBM traffic, which is the largest
single cost. Measured HW rel err ~1e-3.

Inputs are staged host-side in transposed layout ([D, points]) so the
contraction dim lands on SBUF partitions with no on-device transposes.

Schedule shape (engines are in-order FIFOs, so program order matters):
all y input DMAs issue first on the SP ring; the FIRST column half's y
prep is emitted upfront (gpsimd scale + DVE square — ACT's FIFO carries
only the exp stream), the SECOND half's is woven into the middle of the
first half's sweep. Output DMAs are 1 MiB bf16 tiles on the SP ring;
output tiles are triple-buffered so ACT produces one tile ahead of the
ring. Mid-sweep y2-prep matmuls borrow row 0 of main-pool PSUM tiles.

build_nc(repeat=R) emits the whole computation R times into one NEFF
(reps serialized by buffer reuse) — used by test.py to measure the
per-iteration device makespan as a slope, amortizing dispatch overhead.
"""

import numpy as np

import concourse.bass as bass
import concourse.mybir as mybir
import concourse.tile as tile
from concourse import bacc
from concourse.bass_utils import run_bass_kernel_spmd

N_CORES = 8
N, M, D = 8192, 8192, 64
NSH = N // N_CORES  # 1024 x-rows per core

F32 = mybir.dt.float32
F32R = mybir.dt.float32r
BF16 = mybir.dt.bfloat16
AF = mybir.ActivationFunctionType


def build_nc(nsh=NSH, m=M, d=D, use_f32r=True, n_chunk=2048, prep_chunk=1024,
             out_bf16=True, repeat=1):
    """Per-core Bass graph. SPMD: same graph on all 8 cores."""
    nc = bacc.Bacc("TRN2", target_bir_lowering=False)

    odt = BF16 if out_bf16 else F32

    xT = nc.dram_tensor("xT", [d, nsh], F32, kind="ExternalInput")
    yT = nc.dram_tensor("yT", [d, m], F32, kind="ExternalInput")
    lls = nc.dram_tensor("log_lengthscale", [d], F32, kind="ExternalInput")
    los = nc.dram_tensor("log_outputscale", [1], F32, kind="ExternalInput")
    out = nc.dram_tensor("out", [nsh, m], odt, kind="ExternalOutput")

    n_tiles = nsh // 128          # x tiles (output partition dim)
    mm_n = 512                    # moving free dim per matmul (one PSUM bank)
    n_sub = n_chunk // mm_n       # matmuls per ACT chunk
    naux = 2                      # y2 hi + lo rows
    half = m // 2                 # column half per outer iteration
    hc = half // n_chunk

    # fp32r: fp32 bits with the low 12 mantissa bits zeroed; streams at
    # ~1 cycle/row (vs 4 for fp32). Writers into matmul operands must
    # declare float32r output so HW rounds on write (BIR verifier rule).
    def mmi(ap):  # matmul input view
        return ap.bitcast(F32R) if use_f32r else ap

    def mmo(ap):  # rounded-writer output view
        return ap.bitcast(F32R) if use_f32r else ap

    with tile.TileContext(nc) as tc:
        with (
            tc.tile_pool(name="const", bufs=1) as cpool,
            tc.tile_pool(name="big", bufs=1) as bpool,
            tc.tile_pool(name="outp", bufs=3) as opool,
        ):
            # ---- hyperparameters (once; reps reuse them) ----
            lls_sb = cpool.tile([d, 1], F32, tag="lls")
            nc.sync.dma_start(out=lls_sb[:, :], in_=lls[:].rearrange("(d o) -> d o", o=1))
            los_sb = cpool.tile([1, 1], F32, tag="los")
            nc.sync.dma_start(out=los_sb[:, :], in_=los[:].rearrange("(a o) -> a o", o=1))

            invl = cpool.tile([d, 1], F32, tag="invl")
            nc.scalar.activation(invl[:, :], lls_sb[:, :], AF.Exp, scale=-1.0)
            # -0.5 weight vector for the square-reduce matmuls; consumed by
            # f32r matmuls so it needs an f32r-writing producer (copy).
            neghalf_f = cpool.tile([d, 1], F32, tag="neghalf_f")
            nc.vector.memset(neghalf_f[:, :], -0.5)
            neghalf = cpool.tile([d, 1], F32, tag="neghalf")
            nc.vector.tensor_copy(mmo(neghalf[:, :]), neghalf_f[:, :])
            ones11 = cpool.tile([1, 1], F32, tag="ones11")
            nc.vector.memset(ones11[:, :], 1.0)
            ones_rows = cpool.tile([naux, nsh], F32, tag="ones_rows")
            nc.vector.memset(ones_rows[:, :], 1.0)

            y2l_tmp = cpool.tile([1, m], F32, tag="y2l")
            x2row = cpool.tile([1, nsh], F32, tag="x2row")
            bias_sb = cpool.tile([128, n_tiles], F32, tag="bias")

            for _rep in range(repeat):
                # ---- x side: x_aug = [xs; 1; 1] ----
                # Raw DMA lands in a separate tile: every writer of
                # y_aug/x_aug must carry float32r output dtype (BIR
                # fp32r-producer rule).
                x_raw = bpool.tile([d, nsh], F32, tag="x_raw")
                nc.sync.dma_start(out=x_raw[:, :], in_=xT[:, :])
                x_aug = bpool.tile([d + naux, nsh], F32, tag="x_aug")
                nc.vector.tensor_scalar_mul(mmo(x_aug[0:d, :]), x_raw[:, :], invl[:, :])
                xsq = bpool.tile([d, nsh], F32, tag="xsq")
                nc.vector.tensor_mul(mmo(xsq[:, :]), x_aug[0:d, :], x_aug[0:d, :])
                nc.vector.tensor_copy(mmo(x_aug[d : d + naux, :]), ones_rows[:, :])

                y_aug = bpool.tile([d + naux, m], F32, tag="y_aug")

                # x2 bias row + first-half y prep; own PSUM pool, closed
                # before the main pool claims all 8 banks. All y input DMAs
                # issue first on the SP ring so nothing queues behind output
                # DMAs later.
                nchunks = m // prep_chunk
                with (
                    tc.tile_pool(name="yraw_sb", bufs=nchunks) as yrp,
                    tc.tile_pool(name="prep_sb", bufs=2) as psb,
                ):
                  with tc.tile_pool(name="prep_psum", bufs=2, space="PSUM") as pp:
                    y_raws = []
                    for jc in range(0, m, prep_chunk):
                        y_raw = yrp.tile([d, prep_chunk], F32, tag="y_raw")
                        nc.sync.dma_start(out=y_raw[:, :], in_=yT[:, jc : jc + prep_chunk])
                        y_raws.append(y_raw)

                    for j0 in range(0, nsh, mm_n):
                        w = min(mm_n, nsh - j0)
                        ps = pp.tile([1, mm_n], F32, tag="x2ps")
                        nc.tensor.matmul(
                            ps[:, :w], mmi(neghalf[:, :]), mmi(xsq[:, j0 : j0 + w]),
                            start=True, stop=True,
                        )
                        nc.scalar.activation(
                            x2row[:, j0 : j0 + w], ps[:, :w], AF.Identity,
                            bias=los_sb[:, :],
                        )
                    # transpose x2row chunks to per-partition bias cols [128, n_tiles]
                    for i in range(n_tiles):
                        ps = pp.tile([128, 1], F32, tag="biasps")
                        nc.tensor.matmul(
                            ps[:, :], x2row[:, i * 128 : (i + 1) * 128], ones11[:, :],
                            start=True, stop=True,
                        )
                        nc.vector.tensor_copy(bias_sb[:, i : i + 1], ps[:, :])

                    # ---- y prep for the FIRST column half, upfront: fast
                    # head start for the output-DMA stream. Scale on gpsimd,
                    # square + y2-hi copy + y2-lo residual on DVE (ACT's
                    # FIFO carries only the exp stream). DVE writes must
                    # start at partition {0,32,64,96}: row d+1 (partition
                    # 65) goes via a partition-0 tmp + DMA on the scalar
                    # ring. ----
                    for jc in range(0, half, prep_chunk):
                        slc = slice(jc, jc + prep_chunk)
                        y_raw = y_raws[jc // prep_chunk]
                        nc.gpsimd.tensor_scalar_mul(
                            mmo(y_aug[0:d, slc]), y_raw[:, :], invl[:, :]
                        )
                        ysq = psb.tile([d, prep_chunk], F32, tag="ysq")
                        nc.vector.tensor_mul(
                            mmo(ysq[:, :]), y_aug[0:d, slc], y_aug[0:d, slc]
                        )
                        for j0 in range(0, prep_chunk, mm_n):
                            sl = slice(jc + j0, jc + j0 + mm_n)
                            ps = pp.tile([1, mm_n], F32, tag="y2ps")
                            nc.tensor.matmul(
                                ps[:, :], mmi(neghalf[:, :]), mmi(ysq[:, j0 : j0 + mm_n]),
                                start=True, stop=True,
                            )
                            nc.vector.tensor_copy(mmo(y_aug[d : d + 1, sl]), ps[:, :])
                            if use_f32r:
                                nc.vector.tensor_sub(
                                    mmo(y2l_tmp[:, sl]), ps[:, :], y_aug[d : d + 1, sl],
                                )
                            else:
                                nc.vector.memset(y2l_tmp[:, sl], 0.0)
                        nc.scalar.dma_start(
                            out=mmo(y_aug[d + 1 : d + 2, slc]),
                            in_=mmo(y2l_tmp[:, slc]),
                        )

                  # ---- main sweep. The SECOND half's y prep is emitted in
                  # the middle of the first half's sweep: its y2 matmuls slot
                  # into the PE FIFO long before the second half needs them
                  # (no mid-stream DMA gap), and its squares run on DVE,
                  # which idles during the main sweep, so the ACT exp stream
                  # is untouched. ----
                  insert_i = max(0, n_tiles - nchunks // 2)
                  with tc.tile_pool(name="main_psum", bufs=2, space="PSUM") as mp:
                      for h in range(2):
                          for i in range(n_tiles):
                              if h == 0 and i == insert_i:
                                  for jc in range(half, m, prep_chunk):
                                      slc = slice(jc, jc + prep_chunk)
                                      y_raw = y_raws[jc // prep_chunk]
                                      nc.gpsimd.tensor_scalar_mul(
                                          mmo(y_aug[0:d, slc]), y_raw[:, :], invl[:, :]
                                      )
                                      ysq = psb.tile([d, prep_chunk], F32, tag="ysq")
                                      nc.vector.tensor_mul(
                                          mmo(ysq[:, :]), y_aug[0:d, slc], y_aug[0:d, slc]
                                      )
                                      psy = mp.tile([128, n_chunk], F32, tag="mm")
                                      for j0 in range(0, prep_chunk, mm_n):
                                          sl = slice(jc + j0, jc + j0 + mm_n)
                                          pslice = psy[0:1, j0 : j0 + mm_n]
                                          nc.tensor.matmul(
                                              pslice, mmi(neghalf[:, :]),
                                              mmi(ysq[:, j0 : j0 + mm_n]),
                                              start=True, stop=True,
                                          )
                                          nc.vector.tensor_copy(
                                              mmo(y_aug[d : d + 1, sl]), pslice
                                          )
                                          if use_f32r:
                                              nc.vector.tensor_sub(
                                                  mmo(y2l_tmp[:, sl]), pslice,
                                                  y_aug[d : d + 1, sl],
                                              )
                                          else:
                                              nc.vector.memset(y2l_tmp[:, sl], 0.0)
                                      nc.scalar.dma_start(
                                          out=mmo(y_aug[d + 1 : d + 2, slc]),
                                          in_=mmo(y2l_tmp[:, slc]),
                                      )
                              ot = opool.tile([128, half], odt, tag="ot")
                              for j2 in range(hc):
                                  ps = mp.tile([128, n_chunk], F32, tag="mm")
                                  for jj in range(n_sub):
                                      col = h * half + j2 * n_chunk + jj * mm_n
                                      nc.tensor.matmul(
                                          ps[:, jj * mm_n : (jj + 1) * mm_n],
                                          mmi(x_aug[:, i * 128 : (i + 1) * 128]),
                                          mmi(y_aug[:, col : col + mm_n]),
                                          start=True, stop=True,
                                      )
                                  nc.scalar.activation(
                                      ot[:, j2 * n_chunk : (j2 + 1) * n_chunk],
                                      ps[:, :], AF.Exp, bias=bias_sb[:, i : i + 1],
                                  )
                                  if h == 0 and i == 0:
                                      # very first tile: ship each exp chunk
                                      # as its own DMA — the first needs only
                                      # the first two y chunks, starting the
                                      # output stream earlier.
                                      nc.sync.dma_start(
                                          out=out[0:128, j2 * n_chunk : (j2 + 1) * n_chunk],
                                          in_=ot[:, j2 * n_chunk : (j2 + 1) * n_chunk],
                                      )
                              if not (h == 0 and i == 0):
                                  nc.sync.dma_start(
                                      out=out[i * 128 : (i + 1) * 128, h * half : (h + 1) * half],
                                      in_=ot[:, :],
                                  )
    nc.finalize()
    return nc


_NC_CACHE = {}


def _get_nc():
    if "nc" not in _NC_CACHE:
        _NC_CACHE["nc"] = build_nc()
    return _NC_CACHE["nc"]


def stage_inputs(x, y, log_lengthscale, log_outputscale):
    x = np.ascontiguousarray(np.asarray(x, dtype=np.float32))
    y = np.ascontiguousarray(np.asarray(y, dtype=np.float32))
    lls = np.ascontiguousarray(np.asarray(log_lengthscale, dtype=np.float32))
    los = np.ascontiguousarray(np.asarray(log_outputscale, dtype=np.float32))

    yT = np.ascontiguousarray(y.T)  # [D, M]
    in_maps = []
    for c in range(N_CORES):
        xT_c = np.ascontiguousarray(x[c * NSH : (c + 1) * NSH].T)  # [D, NSH]
        in_maps.append(
            {"xT": xT_c, "yT": yT, "log_lengthscale": lls, "log_outputscale": los}
        )
    return in_maps


def assemble_output(out_concat):
    """Map the over-cores-concatenated device output to the final [N, M]."""
    return np.asarray(out_concat).astype(np.float32)


def kernel(x, y, log_lengthscale, log_outputscale):
    in_maps = stage_inputs(x, y, log_lengthscale, log_outputscale)
    res = run_bass_kernel_spmd(_get_nc(), in_maps, core_ids=list(range(N_CORES)))
    return assemble_output(np.concatenate([r["out"] for r in res.results], axis=0))


# revision 10
# speedup vs baseline: 15.8967x; 2.0200x over previous
"""ARD RBF kernel matrix on 8 TRN2 NeuronCores.

out[n, m] = exp(log_outputscale) * exp(-0.5 * sum_d ((x[n,d] - y[m,d]) / l_d)^2)
with l = exp(log_lengthscale).

Per core (rows of x sharded 8-ways), with invl2[d] = exp(-2*log_lengthscale[d]):
the lengthscale is folded into the X side only —
    cross[n, m] = sum_d (x[n,d]*invl2[d]) * y[m,d]
    y2[m] = sum_d y[m,d]^2 * (-0.5*invl2[d])   (matmul, weights -0.5*invl2)
    x2[n] = sum_d x[n,d]^2 * (-0.5*invl2[d]) + log_outputscale
    out[n, m] = Exp(cross + y2 + x2)
so y needs NO elementwise scaling: one DVE f32->f32r rounding copy and
one DVE square per chunk, and the y2 row rides the matmul. One K=97
fp32r matmul per output tile: lhsT = [x*invl2; 1; 0*31; 1],
rhs = [y; y2hi; junk*31; y2lo]. The zero rows in lhsT kill the junk
rows; y2 is split hi+lo (fp32r residual pair, both at DVE-writable
partitions 64/96) so its rounding error stays ~1e-6; x2 + log_os ride
the ACT bias in full f32. exp runs as one ScalarE pass per [128, 2048]
PSUM chunk, written as bf16 (8-bit exponent covers the e^-60-scale
tail; ~0.4% rounding) and upcast to f32 on the host — halves the
output HBM traffic. Measured HW rel err ~1e-3.

Inputs are staged host-side in transposed layout ([D, points]) so the
contraction dim lands on SBUF partitions with no on-device transposes.

Schedule: input DMAs first on the SP ring (x, then y in 4 chunks);
DVE preps x_aug while y chunk 0 lands; per y chunk DVE rounds + squares
and PE reduces y2 (borrowing row 0 of main-pool PSUM tiles — no
separate PSUM pool, no pool barrier); then the 8x16 main matmul /
exp sweep streams row tiles out as 1 MiB bf16 DMAs (first row tile
ships in 0.5 MiB chunks to start the output stream early).

build_nc(repeat=R) emits the whole computation R times into one NEFF
(reps serialized by buffer reuse) — used by test.py to measure the
per-iteration device makespan as a slope, amortizing dispatch overhead.
main_mm/act/outdma=False build ablation variants for differential
timing only (wrong results).
"""

import numpy as np

import concourse.bass as bass
import concourse.mybir as mybir
import concourse.tile as tile
from concourse import bacc
from concourse.bass_utils import run_bass_kernel_spmd

N_CORES = 8
N, M, D = 8192, 8192, 64
NSH = N // N_CORES  # 1024 x-rows per core

F32 = mybir.dt.float32
F32R = mybir.dt.float32r
BF16 = mybir.dt.bfloat16
AF = mybir.ActivationFunctionType


def build_nc(nsh=NSH, m=M, d=D, use_f32r=True, n_chunk=2048, out_bf16=True,
             repeat=1, main_mm=True, act=True, outdma=True, dma_rings=1):
    """Per-core Bass graph. SPMD: same graph on all 8 cores."""
    nc = bacc.Bacc("TRN2", target_bir_lowering=False)

    odt = BF16 if out_bf16 else F32

    xT = nc.dram_tensor("xT", [d, nsh], F32, kind="ExternalInput")
    yT = nc.dram_tensor("yT", [d, m], F32, kind="ExternalInput")
    lls = nc.dram_tensor("log_lengthscale", [d], F32, kind="ExternalInput")
    los = nc.dram_tensor("log_outputscale", [1], F32, kind="ExternalInput")
    out = nc.dram_tensor("out", [nsh, m], odt, kind="ExternalOutput")

    n_tiles = nsh // 128          # x tiles (output partition dim)
    mm_n = 512                    # moving free dim per matmul (one PSUM bank)
    n_sub = n_chunk // mm_n       # matmuls per ACT chunk
    mc = m // n_chunk             # y chunks
    K = 97                        # contraction: 64 data + y2hi@64 + 31 dead + y2lo@96
    ot_w = m // 2                 # output tile width (1 MiB bf16 DMAs)

    def mmi(ap):  # matmul input view
        return ap.bitcast(F32R) if use_f32r else ap

    def mmo(ap):  # rounded-writer output view (BIR fp32r-producer rule)
        return ap.bitcast(F32R) if use_f32r else ap

    # output DMA queue rotation (SP always; optionally ACT HWDGE / SWDGE)
    rings = [nc.sync, nc.scalar, nc.gpsimd][:dma_rings]

    with tile.TileContext(nc) as tc:
        with (
            tc.tile_pool(name="const", bufs=1) as cpool,
            tc.tile_pool(name="xb", bufs=1) as xb,
            tc.tile_pool(name="yraw", bufs=2) as yrp,
            tc.tile_pool(name="ysqp", bufs=2) as ysp,
            tc.tile_pool(name="outp", bufs=3) as opool,
            tc.tile_pool(name="mainps", bufs=2, space="PSUM") as mp,
        ):
            # ---- hyperparameters and constants (once; reps reuse) ----
            lls_sb = cpool.tile([d, 1], F32, tag="lls")
            nc.sync.dma_start(out=lls_sb[:, :], in_=lls[:].rearrange("(d o) -> d o", o=1))
            los_sb = cpool.tile([1, 1], F32, tag="los")
            nc.sync.dma_start(out=los_sb[:, :], in_=los[:].rearrange("(a o) -> a o", o=1))

            invl2 = cpool.tile([d, 1], F32, tag="invl2")  # exp(-2*lls)
            nc.scalar.activation(invl2[:, :], lls_sb[:, :], AF.Exp, scale=-2.0)
            l2 = cpool.tile([d, 1], F32, tag="l2")  # exp(+2*lls)
            nc.scalar.activation(l2[:, :], lls_sb[:, :], AF.Exp, scale=2.0)
            neghalf_f = cpool.tile([d, 1], F32, tag="neghalf_f")
            nc.vector.memset(neghalf_f[:, :], -0.5)
            # y2 reduce weights -0.5*invl2 (ysq holds rounded raw y squares);
            # x2 reduce weights -0.5*exp(2*lls) (xsq holds rounded x*invl2
            # squares) — together they keep sq_dist a perfect square:
            # c + x2 + y2 = -0.5*sum((a/l - y*invl)^2) for a = (x*invl2)_r
            nhi2 = cpool.tile([d, 1], F32, tag="nhi2")
            nc.vector.tensor_mul(mmo(nhi2[:, :]), invl2[:, :], neghalf_f[:, :])
            nhx2 = cpool.tile([d, 1], F32, tag="nhx2")
            nc.vector.tensor_mul(mmo(nhx2[:, :]), l2[:, :], neghalf_f[:, :])
            ones11 = cpool.tile([1, 1], F32, tag="ones11")
            nc.vector.memset(ones11[:, :], 1.0)

            # lhsT aux rows [33, nsh]: 1 at row 64, zeros 65..95, 1 at 96
            aux_rows = cpool.tile([33, nsh], F32, tag="aux_rows")
            nc.vector.memset(aux_rows[:, :], 0.0)
            nc.vector.memset(aux_rows[0:1, :], 1.0)
            nc.vector.memset(aux_rows[32:33, :], 1.0)

            x2row = cpool.tile([1, nsh], F32, tag="x2row")
            bias_sb = cpool.tile([128, n_tiles], F32, tag="bias")

            # y_aug allocated once: rows 64:96 zeroed once so the dead rows
            # 65..95 stay finite forever (0 * junk would NaN the PSUM sum).
            y_aug = cpool.tile([K, m], F32, tag="y_aug")
            nc.vector.memset(y_aug[64:96, :], 0.0)

            for _rep in range(repeat):
                # ---- input DMAs first on the SP ring ----
                x_raw = xb.tile([d, nsh], F32, tag="x_raw")
                nc.sync.dma_start(out=x_raw[:, :], in_=xT[:, :])
                y_raws = []
                for jc in range(mc):
                    y_raw = yrp.tile([d, n_chunk], F32, tag="y_raw")
                    nc.sync.dma_start(
                        out=y_raw[:, :], in_=yT[:, jc * n_chunk : (jc + 1) * n_chunk]
                    )
                    y_raws.append(y_raw)

                # ---- x side (DVE): x_aug = [x*invl2; 1; 0...; 1], xsq = x^2 ----
                x_aug = xb.tile([K, nsh], F32, tag="x_aug")
                nc.vector.tensor_scalar_mul(mmo(x_aug[0:d, :]), x_raw[:, :], invl2[:, :])
                # square the ROUNDED operands (not raw) so sq_dist keeps its
                # perfect-square structure in the rounded values
                xsq = xb.tile([d, nsh], F32, tag="xsq")
                nc.vector.tensor_mul(mmo(xsq[:, :]), x_aug[0:d, :], x_aug[0:d, :])
                nc.vector.tensor_copy(mmo(x_aug[64:K, :]), aux_rows[:, :])

                # ---- x2 row: -0.5*sum x^2*invl2 + log_os (PE reduce + ACT bias) ----
                for j0 in range(0, nsh, mm_n):
                    ps = mp.tile([128, n_chunk], F32, tag="mm")
                    nc.tensor.matmul(
                        ps[0:1, 0:mm_n], mmi(nhx2[:, :]), mmi(xsq[:, j0 : j0 + mm_n]),
                        start=True, stop=True,
                    )
                    nc.scalar.activation(
                        x2row[:, j0 : j0 + mm_n], ps[0:1, 0:mm_n], AF.Identity,
                        bias=los_sb[:, :],
                    )

                # ---- y side per chunk: f32r round, square, y2 reduce ----
                for jc in range(mc):
                    slc = slice(jc * n_chunk, (jc + 1) * n_chunk)
                    y_raw = y_raws[jc]
                    nc.vector.tensor_copy(mmo(y_aug[0:d, slc]), y_raw[:, :])
                    ysq = ysp.tile([d, n_chunk], F32, tag="ysq")
                    nc.vector.tensor_mul(
                        mmo(ysq[:, :]), y_aug[0:d, slc], y_aug[0:d, slc]
                    )
                    psy = mp.tile([128, n_chunk], F32, tag="mm")
                    for j0 in range(0, n_chunk, mm_n):
                        nc.tensor.matmul(
                            psy[0:1, j0 : j0 + mm_n], mmi(nhi2[:, :]),
                            mmi(ysq[:, j0 : j0 + mm_n]),
                            start=True, stop=True,
                        )
                    for j0 in range(0, n_chunk, mm_n):
                        sl = slice(jc * n_chunk + j0, jc * n_chunk + j0 + mm_n)
                        nc.vector.tensor_copy(
                            mmo(y_aug[64:65, sl]), psy[0:1, j0 : j0 + mm_n]
                        )
                        if use_f32r:
                            # lo = exact - rounded hi (fp32r residual pair)
                            nc.vector.tensor_sub(
                                mmo(y_aug[96:97, sl]), psy[0:1, j0 : j0 + mm_n],
                                y_aug[64:65, sl],
                            )
                        else:
                            nc.vector.memset(y_aug[96:97, sl], 0.0)

                # ---- bias transpose: x2row chunks -> per-partition cols ----
                for i in range(n_tiles):
                    ps = mp.tile([128, n_chunk], F32, tag="mm")
                    nc.tensor.matmul(
                        ps[:, 0:1], x2row[:, i * 128 : (i + 1) * 128], ones11[:, :],
                        start=True, stop=True,
                    )
                    nc.vector.tensor_copy(bias_sb[:, i : i + 1], ps[:, 0:1])

                # ---- main sweep: 8 row tiles x 16 matmuls, exp, ship ----
                ndma = 0
                for i in range(n_tiles):
                    for half in range(2):
                        ot = opool.tile([128, ot_w], odt, tag="ot")
                        for j2 in range(ot_w // n_chunk):
                            ps = mp.tile([128, n_chunk], F32, tag="mm")
                            if main_mm:
                                for jj in range(n_sub):
                                    col = half * ot_w + j2 * n_chunk + jj * mm_n
                                    nc.tensor.matmul(
                                        ps[:, jj * mm_n : (jj + 1) * mm_n],
                                        mmi(x_aug[:, i * 128 : (i + 1) * 128]),
                                        mmi(y_aug[:, col : col + mm_n]),
                                        start=True, stop=True,
                                    )
                            else:
                                nc.vector.memset(ps[:, 0:1], 0.0)
                            if act:
                                nc.scalar.activation(
                                    ot[:, j2 * n_chunk : (j2 + 1) * n_chunk],
                                    ps[:, :], AF.Exp, bias=bias_sb[:, i : i + 1],
                                )
                            else:
                                nc.vector.memset(
                                    ot[:, j2 * n_chunk : j2 * n_chunk + 1], 0.0
                                )
                            if outdma and i == 0 and half == 0:
                                # first tile: ship per-chunk to start the
                                # output stream as early as possible
                                rings[ndma % len(rings)].dma_start(
                                    out=out[0:128, j2 * n_chunk : (j2 + 1) * n_chunk],
                                    in_=ot[:, j2 * n_chunk : (j2 + 1) * n_chunk],
                                )
                                ndma += 1
                        if outdma and not (i == 0 and half == 0):
                            rings[ndma % len(rings)].dma_start(
                                out=out[i * 128 : (i + 1) * 128,
                                        half * ot_w : (half + 1) * ot_w],
                                in_=ot[:, :],
                            )
                            ndma += 1
    nc.finalize()
    return nc


_NC_CACHE = {}


def _get_nc():
    if "nc" not in _NC_CACHE:
        _NC_CACHE["nc"] = build_nc()
    return _NC_CACHE["nc"]


def stage_inputs(x, y, log_lengthscale, log_outputscale):
    x = np.ascontiguousarray(np.asarray(x, dtype=np.float32))
    y = np.ascontiguousarray(np.asarray(y, dtype=np.float32))
    lls = np.ascontiguousarray(np.asarray(log_lengthscale, dtype=np.float32))
    los = np.ascontiguousarray(np.asarray(log_outputscale, dtype=np.float32))

    yT = np.ascontiguousarray(y.T)  # [D, M]
    in_maps = []
    for c in range(N_CORES):
        xT_c = np.ascontiguousarray(x[c * NSH : (c + 1) * NSH].T)  # [D, NSH]
        in_maps.append(
            {"xT": xT_c, "yT": yT, "log_lengthscale": lls, "log_outputscale": los}
        )
    return in_maps


def assemble_output(out_concat):
    """Map the over-cores-concatenated device output to the final [N, M]."""
    return np.asarray(out_concat).astype(np.float32)


def kernel(x, y, log_lengthscale, log_outputscale):
    in_maps = stage_inputs(x, y, log_lengthscale, log_outputscale)
    res = run_bass_kernel_spmd(_get_nc(), in_maps, core_ids=list(range(N_CORES)))
    return assemble_output(np.concatenate([r["out"] for r in res.results], axis=0))
